# revision 40
# baseline (speedup 1.0000x reference)
"""Adaptive-style-attention (AdaAttN-like) Trainium2 kernel, 8 NeuronCores.

Math (per batch b, with N = M = 64*64 = 4096 pixels, C = Ck = 256):
  Fq = Wf @ content_key[b] + bf          # [C, N]   (q^T)
  G  = Wg @ style_key[b]   + bg          # [C, M]   (k)
  Hv = Wh @ style[b]       + bh          # [C, M];  V = Hv^T  [M, C]
  S  = softmax_m(q @ k)                  # [N, M]
  mean = S @ V ; e2 = S @ V^2            # [N, C]
  std  = sqrt(relu(e2 - mean^2))
  out  = std * mvn(content[b]) + mean    # [C, N] layout

Sharding: 8 cores = batch(4) x query-halves(2). Each core computes its
2048 query rows against the full 4096 style pixels of its batch.

Everything is computed transposed ([c, n] / [m, n] layouts) so no
on-chip transposes are needed:
  logits^T tile [m=128, n=512] = G_chunk.T @ Fq_chunk   (K = c)
  P^T = exp(logits^T - SHIFT)  (global shift; logits ~ N(0, 256); the
        actual global max logit is ~97, exp(97-48) fits fp32 easily)
  mean^T [c, n] += (V[m, c])-as-lhsT @ P^T  (K = m), PSUM-accumulated
  rowsum via elementwise P accumulation on DVE + one GpSimd
        partition_all_reduce per n-macro (frees the TensorengIne), the
        all-reduce also broadcasts, so 1/rowsum needs no extra matmul.
  out = std * normc + mean.

All matmuls run in float32r: on TRN2 silicon f32r streams the moving
operand at 2 cycles/row (~400 ns per 128x128x512 matmul) with the fused
4-byte weight load fully hidden, and carries ~14-bit mantissa accuracy
(probe: rms 2.3e-3 on K=256 N(0,16^2) logits, 5x better than tf32
emulation).  bf16 matmuls measure the SAME ~380-400 ns here (the per-
matmul LDWEIGHTS cannot hide under a 216 ns stream and FWL is disabled
in this toolchain), so bf16 gives no speed advantage and costs softmax
accuracy - f32r everywhere is optimal.  The BIR verifier requires f32r
matmul operands to be produced rounded, hence compute-engine rounding
copies on every DMA-staged conv input.
"""
import os
import numpy as np

import concourse.bass as bass
import concourse.mybir as mybir
import concourse.tile as tile
from concourse import bacc
from concourse.bass_utils import run_bass_kernel_spmd

B, C, HW = 4, 256, 64 * 64          # N = M = HW
NSH = HW // 2                        # queries per core = 2048
SHIFT = 48.0
EPS = 1e-5
F32 = mybir.dt.float32
F32R = mybir.dt.float32r
BF16 = mybir.dt.bfloat16
AF = mybir.ActivationFunctionType

_last_result = {}


def _build_nc() -> bass.Bass:
    nc = bacc.Bacc("TRN2", target_bir_lowering=False)
    ck = nc.dram_tensor("ck", [128, 2, NSH], F32, kind="ExternalInput")        # content_key shard [p, kchunk, n]
    ct = nc.dram_tensor("ct", [128, 2, 2, NSH], F32, kind="ExternalInput")     # content (rotated) [p, half, kchunk, n]
    sk = nc.dram_tensor("sk", [128, 2, 2, NSH], F32, kind="ExternalInput")     # style_key [p, half, kchunk, m]
    st = nc.dram_tensor("st", [128, 2, 2, NSH], F32, kind="ExternalInput")     # style     [p, half, kchunk, m]
    wf = nc.dram_tensor("wf", [128, 2, C], F32, kind="ExternalInput")          # Wf^T [p(ch), chunk, c_out]
    wg = nc.dram_tensor("wg", [128, 2, C], F32, kind="ExternalInput")
    wh = nc.dram_tensor("wh", [128, 2, C], F32, kind="ExternalInput")
    bfb = nc.dram_tensor("bfb", [128, 2], F32, kind="ExternalInput")           # bf [p, c-chunk]
    bgb = nc.dram_tensor("bgb", [128, 2], F32, kind="ExternalInput")
    bhb = nc.dram_tensor("bhb", [128, C], F32, kind="ExternalInput")           # bh broadcast over partitions
    out = nc.dram_tensor("out", [128, 2, NSH], F32, kind="ExternalOutput")     # [p, c-chunk, n]

    with tile.TileContext(nc) as tc:
        _emit(nc, tc, ck, ct, sk, st, wf, wg, wh, bfb, bgb, bhb, out)
    nc.compile()
    return nc


def _emit(nc, tc, ck, ct, sk, st, wf, wg, wh, bfb, bgb, bhb, out):
    from contextlib import ExitStack

    NM = 4          # n macro tiles of 512 within the 2048-query shard
    MT = 32         # m tiles of 128 within 4096 style pixels
    NW = 512

    with ExitStack() as persist:
        consts = persist.enter_context(tc.tile_pool(name="consts", bufs=1))
        fq_p = persist.enter_context(tc.tile_pool(name="fq", bufs=1))
        g_p = persist.enter_context(tc.tile_pool(name="g", bufs=1))
        v_p = persist.enter_context(tc.tile_pool(name="v", bufs=1))
        v2_p = persist.enter_context(tc.tile_pool(name="v2", bufs=1))
        nrm_p = persist.enter_context(tc.tile_pool(name="nrm", bufs=1))

        negshift = consts.tile([128, 1], F32)
        nc.vector.memset(negshift[:], -SHIFT)

        fq_s = fq_p.tile([128, 2, NSH], F32R)     # [p, c-chunk, n]
        g_s = g_p.tile([128, 2, HW], F32R)        # [p, c-chunk, m]
        v_s = v_p.tile([128, MT, C], F32R)        # [p(m), m-tile, c]
        v2_s = v2_p.tile([128, MT, C], F32R)
        nrm_s = nrm_p.tile([128, 2, NSH], F32)    # normalized content [p, c-chunk, n]

        # ---------------- staging + convs (stats deferred) ----------------
        # Conv inputs stream through small chunk pools: DMA (f32) -> ACT
        # rounding copy (-> f32r) -> matmuls.  G and V convs interleave so
        # both style tensors stream concurrently.  Content stats are emitted
        # later (inside the first attention n-macro) so their DMA + reduce
        # work does not compete with kernel startup.
        ct_pool = persist.enter_context(tc.tile_pool(name="ctsh", bufs=1))
        schk = persist.enter_context(tc.tile_pool(name="schk", bufs=2))
        sml = persist.enter_context(tc.tile_pool(name="sml", bufs=24))
        prt = persist.enter_context(tc.tile_pool(name="prt", bufs=1))
        sqd = persist.enter_context(tc.tile_pool(name="sqd", bufs=2))

        def emit_stats():
            ct_sh = ct_pool.tile([128, 2, NSH], F32)
            nc.sync.dma_start(ct_sh[:], ct[:, 0, :, :])
            parts_s = prt.tile([128, 2, 8], F32)    # Σx partials   [p, k, col]
            parts_q = prt.tile([128, 2, 8], F32)    # Σx² partials
            col = 0
            for c in range(NM):
                for k in range(2):
                    sl = ct_sh[:, k, c * NW:(c + 1) * NW]
                    nc.vector.reduce_sum(parts_s[:, k, col:col + 1], sl,
                                         axis=mybir.AxisListType.X)
                    dump = sqd.tile([128, NW], F32, tag="sqd", name=f"sqa{c}_{k}")
                    nc.scalar.activation(dump[:], sl, AF.Square,
                                         accum_out=parts_q[:, k, col:col + 1])
                col += 1
            for c in range(NM):
                t = schk.tile([128, 2, NW], F32, tag="schk", name=f"cth{c}")
                nc.sync.dma_start(t[:], ct[:, 1, :, c * NW:(c + 1) * NW])
                for k in range(2):
                    nc.vector.reduce_sum(parts_s[:, k, col:col + 1], t[:, k, :],
                                         axis=mybir.AxisListType.X)
                    dump = sqd.tile([128, NW], F32, tag="sqd", name=f"sqb{c}_{k}")
                    nc.scalar.activation(dump[:], t[:, k, :], AF.Square,
                                         accum_out=parts_q[:, k, col:col + 1])
                col += 1
            for k in range(2):
                s_all = sml.tile([128, 1], F32, tag="sml", name=f"sa{k}")
                ss_all = sml.tile([128, 1], F32, tag="sml", name=f"ssa{k}")
                nc.vector.reduce_sum(s_all[:], parts_s[:, k, :], axis=mybir.AxisListType.X)
                nc.vector.reduce_sum(ss_all[:], parts_q[:, k, :], axis=mybir.AxisListType.X)
                m_t = sml.tile([128, 1], F32, tag="sml", name=f"m{k}")
                nc.vector.tensor_scalar_mul(m_t[:], s_all[:], 1.0 / HW)
                msq = sml.tile([128, 1], F32, tag="sml", name=f"msq{k}")
                nc.vector.tensor_mul(msq[:], s_all[:], m_t[:])       # (Σx)²/N
                var = sml.tile([128, 1], F32, tag="sml", name=f"va{k}")
                nc.vector.tensor_sub(var[:], ss_all[:], msq[:])
                nc.vector.tensor_scalar_mul(var[:], var[:], 1.0 / (HW - 1))
                nc.vector.tensor_scalar_add(var[:], var[:], EPS)
                sd = sml.tile([128, 1], F32, tag="sml", name=f"sd{k}")
                nc.scalar.sqrt(sd[:], var[:])
                rstd = sml.tile([128, 1], F32, tag="sml", name=f"rs{k}")
                nc.vector.reciprocal(rstd[:], sd[:])
                nmr = sml.tile([128, 1], F32, tag="sml", name=f"nm{k}")
                nc.vector.tensor_mul(nmr[:], m_t[:], rstd[:])
                nc.vector.tensor_scalar_mul(nmr[:], nmr[:], -1.0)
                nc.scalar.activation(nrm_s[:, k, :], ct_sh[:, k, :],
                                     AF.Identity, bias=nmr[:], scale=rstd[:])

        with ExitStack() as stg:
            wcon = stg.enter_context(tc.tile_pool(name="wcon", bufs=1))
            chk = stg.enter_context(tc.tile_pool(name="chk", bufs=4))
            chkr = stg.enter_context(tc.tile_pool(name="chkr", bufs=4))
            vtmp = stg.enter_context(tc.tile_pool(name="vtmp", bufs=3))
            cps = stg.enter_context(tc.tile_pool(name="cpsum", bufs=4, space="PSUM"))
            vps = stg.enter_context(tc.tile_pool(name="vpsum", bufs=4, space="PSUM"))

            wf_s = wcon.tile([128, 2, C], F32)
            wg_s = wcon.tile([128, 2, C], F32)
            wh_s = wcon.tile([128, 2, C], F32)
            bf_s = wcon.tile([128, 2], F32)
            bg_s = wcon.tile([128, 2], F32)
            bh_s = wcon.tile([128, C], F32)
            nc.sync.dma_start(wf_s[:], wf[:])
            nc.sync.dma_start(wg_s[:], wg[:])
            nc.sync.dma_start(wh_s[:], wh[:])
            nc.sync.dma_start(bf_s[:], bfb[:])
            nc.sync.dma_start(bg_s[:], bgb[:])
            nc.sync.dma_start(bh_s[:], bhb[:])
            wf_r = wcon.tile([128, 2, C], F32R)
            wg_r = wcon.tile([128, 2, C], F32R)
            wh_r = wcon.tile([128, 2, C], F32R)
            nc.vector.tensor_copy(wf_r[:], wf_s[:])
            nc.vector.tensor_copy(wg_r[:], wg_s[:])
            nc.vector.tensor_copy(wh_r[:], wh_s[:])

            def staged_r(dram_slice, name):
                """DMA a [128, 2, NW] chunk then round it into an f32r tile."""
                t = chk.tile([128, 2, NW], F32, tag="chk", name=name + "_f")
                nc.sync.dma_start(t[:], dram_slice)
                tr = chkr.tile([128, 2, NW], F32R, tag="chkr", name=name + "_r")
                nc.scalar.copy(tr[:], t[:])
                return tr

            wfr = wf_r[:]
            wgr = wg_r[:]
            whr = wh_r[:]

            # Fq conv (content_key shard): out[c2, n] = WfT.T @ ck + bf
            for nm in range(NM):
                tr = staged_r(ck[:, :, nm * NW:(nm + 1) * NW], f"ck{nm}")
                for c2 in range(2):
                    ps = cps.tile([128, NW], F32, tag="cps")
                    for k in range(2):
                        nc.tensor.matmul(
                            ps[:],
                            wfr[:, k, c2 * 128:(c2 + 1) * 128],
                            tr[:, k, :],
                            start=(k == 0), stop=(k == 1))
                    nc.scalar.activation(fq_s[:, c2, nm * NW:(nm + 1) * NW], ps[:],
                                         AF.Identity, bias=bf_s[:, c2:c2 + 1], scale=1.0)

            # G conv (style_key) and V conv (style), interleaved per chunk:
            # G: [c2, m] = WgT.T @ sk + bg ;  V[m, c] = st.T @ WhT + bh, V2 = V^2
            for h in range(2):
                for lm in range(NM):
                    mm = h * NM + lm
                    tg = staged_r(sk[:, h, :, lm * NW:(lm + 1) * NW], f"sk{mm}")
                    tv = staged_r(st[:, h, :, lm * NW:(lm + 1) * NW], f"st{mm}")
                    for c2 in range(2):
                        ps = cps.tile([128, NW], F32, tag="cps")
                        for k in range(2):
                            nc.tensor.matmul(
                                ps[:],
                                wgr[:, k, c2 * 128:(c2 + 1) * 128],
                                tg[:, k, :],
                                start=(k == 0), stop=(k == 1))
                        nc.scalar.activation(g_s[:, c2, mm * NW:(mm + 1) * NW], ps[:],
                                             AF.Identity, bias=bg_s[:, c2:c2 + 1], scale=1.0)
                    for sub in range(NM):
                        mt = mm * NM + sub
                        ps = vps.tile([128, C], F32, tag="vps")
                        for k in range(2):
                            nc.tensor.matmul(
                                ps[:],
                                tv[:, k, sub * 128:(sub + 1) * 128],
                                whr[:, k, :],
                                start=(k == 0), stop=(k == 1))
                        vt = vtmp.tile([128, C], F32, tag="vt", name=f"vt{mt}", bufs=2)
                        nc.vector.tensor_add(vt[:], ps[:], bh_s[:])
                        nc.vector.tensor_copy(v_s[:, mt, :], vt[:])
                        nc.vector.tensor_mul(v2_s[:, mt, :], vt[:], vt[:])

        # ---------------- flash attention inner loops ----------------
        from concourse import bass_isa

        with ExitStack() as inner:
            pt_pool = inner.enter_context(tc.tile_pool(name="pt", bufs=4))
            wrk = inner.enter_context(tc.tile_pool(name="wrk", bufs=14))
            accp = inner.enter_context(tc.tile_pool(name="accp", bufs=2))
            outp = inner.enter_context(tc.tile_pool(name="outp", bufs=4))
            lps = inner.enter_context(tc.tile_pool(name="lpsum", bufs=2, space="PSUM"))
            aps = inner.enter_context(tc.tile_pool(name="apsum", bufs=6, space="PSUM"))

            for nm in range(NM):
                nsl = slice(nm * NW, (nm + 1) * NW)
                mean_ps = [aps.tile([128, NW], F32, tag="acc", name=f"mean_ps{nm}_{i}")
                           for i in range(2)]
                e2_ps = [aps.tile([128, NW], F32, tag="acc", name=f"e2_ps{nm}_{i}")
                         for i in range(2)]
                acc = accp.tile([128, NW], F32, tag="acc", name=f"pacc{nm}")
                for mt in range(MT):
                    msl = slice(mt * 128, (mt + 1) * 128)
                    ps_l = lps.tile([128, NW], F32, tag="log")
                    for k in range(2):
                        nc.tensor.matmul(ps_l[:],
                                         g_s[:, k, msl],
                                         fq_s[:, k, nsl],
                                         start=(k == 0), stop=(k == 1))
                    pt = pt_pool.tile([128, NW], F32R, tag="pt")
                    nc.scalar.activation(pt[:], ps_l[:], AF.Exp,
                                         bias=negshift[:], scale=1.0)
                    first, last = (mt == 0), (mt == MT - 1)
                    for c2 in range(2):
                        nc.tensor.matmul(mean_ps[c2][:],
                                         v_s[:, mt, c2 * 128:(c2 + 1) * 128],
                                         pt[:], start=first, stop=last)
                    for c2 in range(2):
                        nc.tensor.matmul(e2_ps[c2][:],
                                         v2_s[:, mt, c2 * 128:(c2 + 1) * 128],
                                         pt[:], start=first, stop=last)
                    # partition-wise accumulate P for the softmax denominator
                    if first:
                        nc.vector.tensor_copy(acc[:], pt[:])
                    else:
                        nc.vector.tensor_add(acc[:], acc[:], pt[:])



                if nm == 0:
                    # Content stats slot into the nm=0 window: DMAs queue
                    # behind the style tensors; reduces fill engine slack.
                    emit_stats()

                # Eagerly drain the accumulator PSUM banks to SBUF (on ACT)
                # so the next n-macro's matmuls never wait on the reciprocal
                # chain and DVE keeps its slack for the epilogue.
                mean_sb, e2_sb = [], []
                for c2 in range(2):
                    t = wrk.tile([128, NW], F32, tag="wrk", name=f"msb{nm}_{c2}")
                    nc.scalar.copy(t[:], mean_ps[c2][:])
                    mean_sb.append(t)
                    t = wrk.tile([128, NW], F32, tag="wrk", name=f"esb{nm}_{c2}")
                    nc.scalar.copy(t[:], e2_ps[c2][:])
                    e2_sb.append(t)

                # epilogue for this n-macro: rowsum = partition-reduce(acc),
                # broadcast to all partitions by the GpSimd daisy chain.
                rsb = wrk.tile([128, NW], F32, tag="wrk", name=f"rsb{nm}")
                nc.gpsimd.partition_all_reduce(rsb[:], acc[:], channels=128,
                                               reduce_op=bass_isa.ReduceOp.add)
                recip_b = wrk.tile([128, NW], F32, tag="wrk", name=f"rcp{nm}")
                nc.vector.reciprocal(recip_b[:], rsb[:])
                meanN, e2N, var = [], [], []
                for c2 in range(2):
                    t = wrk.tile([128, NW], F32, tag="wrk", name=f"mn{nm}_{c2}")
                    nc.vector.tensor_mul(t[:], mean_sb[c2][:], recip_b[:])
                    meanN.append(t)
                    t = wrk.tile([128, NW], F32, tag="wrk", name=f"e2{nm}_{c2}")
                    nc.vector.tensor_mul(t[:], e2_sb[c2][:], recip_b[:])
                    e2N.append(t)
                for c2 in range(2):
                    sq = wrk.tile([128, NW], F32, tag="wrk", name=f"sq{nm}_{c2}")
                    nc.vector.tensor_mul(sq[:], meanN[c2][:], meanN[c2][:])
                    v = wrk.tile([128, NW], F32, tag="wrk", name=f"vr{nm}_{c2}")
                    nc.vector.tensor_sub(v[:], e2N[c2][:], sq[:])
                    nc.vector.tensor_scalar_max(v[:], v[:], 0.0)
                    var.append(v)
                stds = []
                for c2 in range(2):
                    s = wrk.tile([128, NW], F32, tag="wrk", name=f"sd{nm}_{c2}")
                    nc.scalar.sqrt(s[:], var[c2][:])
                    stds.append(s)
                for c2 in range(2):
                    ot = outp.tile([128, NW], F32, tag="out", name=f"ot{nm}_{c2}")
                    nc.vector.tensor_mul(ot[:], stds[c2][:], nrm_s[:, c2, nsl])
                    nc.vector.tensor_add(ot[:], ot[:], meanN[c2][:])
                    nc.sync.dma_start(out[:, c2, nsl], ot[:])


def kernel(content, style, content_key, style_key, Wf, bf, Wg, bg, Wh, bh):
    content = np.ascontiguousarray(np.asarray(content, dtype=np.float32))
    style = np.ascontiguousarray(np.asarray(style, dtype=np.float32))
    content_key = np.ascontiguousarray(np.asarray(content_key, dtype=np.float32))
    style_key = np.ascontiguousarray(np.asarray(style_key, dtype=np.float32))
    Wf = np.asarray(Wf, dtype=np.float32)
    Wg = np.asarray(Wg, dtype=np.float32)
    Wh = np.asarray(Wh, dtype=np.float32)
    bf = np.asarray(bf, dtype=np.float32)
    bg = np.asarray(bg, dtype=np.float32)
    bh = np.asarray(bh, dtype=np.float32)

    def wlay(W):  # [O, C] -> [128, 2, 256] with [p, k, c_out] = W[c_out, k*128+p]
        return np.ascontiguousarray(W.T.reshape(2, 128, C).transpose(1, 0, 2))

    def blay(b):  # [256] -> [128, 2]
        return np.ascontiguousarray(b.reshape(2, 128).T)

    def big_lay(x):  # [256, 4096] -> [128, 2(h), 2(k), 2048]
        return np.ascontiguousarray(
            x.reshape(2, 128, 2, NSH).transpose(1, 2, 0, 3))

    wf_l, wg_l, wh_l = wlay(Wf), wlay(Wg), wlay(Wh)
    bf_l, bg_l = blay(bf), blay(bg)
    bh_b = np.ascontiguousarray(np.broadcast_to(bh, (128, C)))

    in_maps = []
    for core in range(8):
        b, half = core // 2, core % 2
        off = half * NSH
        ctb = content[b].reshape(C, HW)
        ct_rot = np.concatenate([ctb[:, off:], ctb[:, :off]], axis=1) if off else ctb
        ck_sh = content_key[b].reshape(C, HW)[:, off:off + NSH]
        in_maps.append({
            "ck": np.ascontiguousarray(ck_sh.reshape(2, 128, NSH).transpose(1, 0, 2)),
            "ct": big_lay(ct_rot),
            "sk": big_lay(style_key[b].reshape(C, HW)),
            "st": big_lay(style[b].reshape(C, HW)),
            "wf": wf_l, "wg": wg_l, "wh": wh_l,
            "bfb": bf_l, "bgb": bg_l, "bhb": bh_b,
        })

    nc = _build_nc()
    trace = bool(os.environ.get("KERNEL_TRACE"))
    res = run_bass_kernel_spmd(nc, in_maps, core_ids=list(range(8)), trace=trace)
    _last_result.clear()
    _last_result["exec_time_ns"] = res.exec_time_ns
    _last_result["trace"] = res.instructions_and_trace

    outp = np.empty((B, C, HW), dtype=np.float32)
    for core in range(8):
        b, half = core // 2, core % 2
        o = res.results[core]["out"]          # [128, 2, NSH]
        outp[b, :, half * NSH:(half + 1) * NSH] = (
            o.transpose(1, 0, 2).reshape(C, NSH))
    return outp.reshape(B, C, 64, 64)


# revision 41
# speedup vs baseline: 1.1095x; 1.1095x over previous
"""Adaptive-style-attention (AdaAttN-like) Trainium2 kernel, 8 NeuronCores.

Math (per batch b, with N = M = 64*64 = 4096 pixels, C = Ck = 256):
  Fq = Wf @ content_key[b] + bf          # [C, N]   (q^T)
  G  = Wg @ style_key[b]   + bg          # [C, M]   (k)
  Hv = Wh @ style[b]       + bh          # [C, M];  V = Hv^T  [M, C]
  S  = softmax_m(q @ k)                  # [N, M]
  mean = S @ V ; e2 = S @ V^2            # [N, C]
  std  = sqrt(relu(e2 - mean^2))
  out  = std * mvn(content[b]) + mean    # [C, N] layout

Sharding: 8 cores = batch(4) x query-halves(2). Each core computes its
2048 query rows against the full 4096 style pixels of its batch.

Everything is computed transposed ([c, n] / [m, n] layouts) so no
on-chip transposes are needed:
  logits^T tile [m=128, n=512] = G_chunk.T @ Fq_chunk   (K = c)
  P^T = exp(logits^T - SHIFT)  (global shift; logits ~ N(0, 256); the
        actual global max logit is ~97, exp(97-48) fits fp32 easily)
  mean^T [c, n] += (V[m, c])-as-lhsT @ P^T  (K = m), PSUM-accumulated
  rowsum via elementwise P accumulation on DVE + one GpSimd
        partition_all_reduce per n-macro (frees the TensorengIne), the
        all-reduce also broadcasts, so 1/rowsum needs no extra matmul.
  out = std * normc + mean.

All matmuls run in float32r: on TRN2 silicon f32r streams the moving
operand at 2 cycles/row (~400 ns per 128x128x512 matmul) with the fused
4-byte weight load fully hidden, and carries ~14-bit mantissa accuracy
(probe: rms 2.3e-3 on K=256 N(0,16^2) logits, 5x better than tf32
emulation).  bf16 matmuls measure the SAME ~380-400 ns here (the per-
matmul LDWEIGHTS cannot hide under a 216 ns stream and FWL is disabled
in this toolchain), so bf16 gives no speed advantage and costs softmax
accuracy - f32r everywhere is optimal.  The BIR verifier requires f32r
matmul operands to be produced rounded, hence compute-engine rounding
copies on every DMA-staged conv input.
"""
import os
import numpy as np

import concourse.bass as bass
import concourse.mybir as mybir
import concourse.tile as tile
from concourse import bacc
from concourse.bass_utils import run_bass_kernel_spmd

B, C, HW = 4, 256, 64 * 64          # N = M = HW
NSH = HW // 2                        # queries per core = 2048
SHIFT = 48.0
EPS = 1e-5
F32 = mybir.dt.float32
F32R = mybir.dt.float32r
BF16 = mybir.dt.bfloat16
AF = mybir.ActivationFunctionType

_last_result = {}


def _build_nc() -> bass.Bass:
    nc = bacc.Bacc("TRN2", target_bir_lowering=False)
    ck = nc.dram_tensor("ck", [128, 2, NSH], F32, kind="ExternalInput")        # content_key shard [p, kchunk, n]
    ct = nc.dram_tensor("ct", [128, 2, 2, NSH], F32, kind="ExternalInput")     # content (rotated) [p, half, kchunk, n]
    sk = nc.dram_tensor("sk", [128, 2, 2, NSH], F32, kind="ExternalInput")     # style_key [p, half, kchunk, m]
    st = nc.dram_tensor("st", [128, 2, 2, NSH], F32, kind="ExternalInput")     # style     [p, half, kchunk, m]
    wf = nc.dram_tensor("wf", [128, 2, C], F32, kind="ExternalInput")          # Wf^T [p(ch), chunk, c_out]
    wg = nc.dram_tensor("wg", [128, 2, C], F32, kind="ExternalInput")
    wh = nc.dram_tensor("wh", [128, 2, C], F32, kind="ExternalInput")
    bfb = nc.dram_tensor("bfb", [128, 2], F32, kind="ExternalInput")           # bf [p, c-chunk]
    bgb = nc.dram_tensor("bgb", [128, 2], F32, kind="ExternalInput")
    bhb = nc.dram_tensor("bhb", [128, C], F32, kind="ExternalInput")           # bh broadcast over partitions
    out = nc.dram_tensor("out", [128, 2, NSH], F32, kind="ExternalOutput")     # [p, c-chunk, n]

    with tile.TileContext(nc) as tc:
        _emit(nc, tc, ck, ct, sk, st, wf, wg, wh, bfb, bgb, bhb, out)
    nc.compile()
    return nc


def _emit(nc, tc, ck, ct, sk, st, wf, wg, wh, bfb, bgb, bhb, out):
    from contextlib import ExitStack

    NM = 4          # n macro tiles of 512 within the 2048-query shard
    MT = 32         # m tiles of 128 within 4096 style pixels
    NW = 512

    with ExitStack() as persist:
        consts = persist.enter_context(tc.tile_pool(name="consts", bufs=1))
        fq_p = persist.enter_context(tc.tile_pool(name="fq", bufs=1))
        g_p = persist.enter_context(tc.tile_pool(name="g", bufs=1))
        v_p = persist.enter_context(tc.tile_pool(name="v", bufs=1))
        v2_p = persist.enter_context(tc.tile_pool(name="v2", bufs=1))
        nrm_p = persist.enter_context(tc.tile_pool(name="nrm", bufs=1))

        negshift = consts.tile([128, 1], F32)
        nc.vector.memset(negshift[:], -SHIFT)

        fq_s = fq_p.tile([128, 2, NSH], F32R)     # [p, c-chunk, n]
        g_s = g_p.tile([128, 2, HW], F32R)        # [p, c-chunk, m]
        v_s = v_p.tile([128, MT, C], F32R)        # [p(m), m-tile, c]
        v2_s = v2_p.tile([128, MT, C], F32R)
        nrm_s = nrm_p.tile([128, 2, NSH], F32)    # normalized content [p, c-chunk, n]

        # ---------------- staging + convs (stats deferred) ----------------
        # Conv inputs stream through small chunk pools: DMA (f32) -> ACT
        # rounding copy (-> f32r) -> matmuls.  G and V convs interleave so
        # both style tensors stream concurrently.  Content stats are emitted
        # later (inside the first attention n-macro) so their DMA + reduce
        # work does not compete with kernel startup.
        ct_pool = persist.enter_context(tc.tile_pool(name="ctsh", bufs=1))
        schk = persist.enter_context(tc.tile_pool(name="schk", bufs=2))
        sml = persist.enter_context(tc.tile_pool(name="sml", bufs=24))
        prt = persist.enter_context(tc.tile_pool(name="prt", bufs=1))
        sqd = persist.enter_context(tc.tile_pool(name="sqd", bufs=2))

        def emit_stats():
            ct_sh = ct_pool.tile([128, 2, NSH], F32)
            nc.sync.dma_start(ct_sh[:], ct[:, 0, :, :])
            parts_s = prt.tile([128, 2, 8], F32)    # Σx partials   [p, k, col]
            parts_q = prt.tile([128, 2, 8], F32)    # Σx² partials
            col = 0
            for c in range(NM):
                for k in range(2):
                    sl = ct_sh[:, k, c * NW:(c + 1) * NW]
                    nc.vector.reduce_sum(parts_s[:, k, col:col + 1], sl,
                                         axis=mybir.AxisListType.X)
                    dump = sqd.tile([128, NW], F32, tag="sqd", name=f"sqa{c}_{k}")
                    nc.scalar.activation(dump[:], sl, AF.Square,
                                         accum_out=parts_q[:, k, col:col + 1])
                col += 1
            for c in range(NM):
                t = schk.tile([128, 2, NW], F32, tag="schk", name=f"cth{c}")
                nc.sync.dma_start(t[:], ct[:, 1, :, c * NW:(c + 1) * NW])
                for k in range(2):
                    nc.vector.reduce_sum(parts_s[:, k, col:col + 1], t[:, k, :],
                                         axis=mybir.AxisListType.X)
                    dump = sqd.tile([128, NW], F32, tag="sqd", name=f"sqb{c}_{k}")
                    nc.scalar.activation(dump[:], t[:, k, :], AF.Square,
                                         accum_out=parts_q[:, k, col:col + 1])
                col += 1
            for k in range(2):
                s_all = sml.tile([128, 1], F32, tag="sml", name=f"sa{k}")
                ss_all = sml.tile([128, 1], F32, tag="sml", name=f"ssa{k}")
                nc.vector.reduce_sum(s_all[:], parts_s[:, k, :], axis=mybir.AxisListType.X)
                nc.vector.reduce_sum(ss_all[:], parts_q[:, k, :], axis=mybir.AxisListType.X)
                m_t = sml.tile([128, 1], F32, tag="sml", name=f"m{k}")
                nc.vector.tensor_scalar_mul(m_t[:], s_all[:], 1.0 / HW)
                msq = sml.tile([128, 1], F32, tag="sml", name=f"msq{k}")
                nc.vector.tensor_mul(msq[:], s_all[:], m_t[:])       # (Σx)²/N
                var = sml.tile([128, 1], F32, tag="sml", name=f"va{k}")
                nc.vector.tensor_sub(var[:], ss_all[:], msq[:])
                nc.vector.tensor_scalar_mul(var[:], var[:], 1.0 / (HW - 1))
                nc.vector.tensor_scalar_add(var[:], var[:], EPS)
                sd = sml.tile([128, 1], F32, tag="sml", name=f"sd{k}")
                nc.scalar.sqrt(sd[:], var[:])
                rstd = sml.tile([128, 1], F32, tag="sml", name=f"rs{k}")
                nc.vector.reciprocal(rstd[:], sd[:])
                nmr = sml.tile([128, 1], F32, tag="sml", name=f"nm{k}")
                nc.vector.tensor_mul(nmr[:], m_t[:], rstd[:])
                nc.vector.tensor_scalar_mul(nmr[:], nmr[:], -1.0)
                nc.scalar.activation(nrm_s[:, k, :], ct_sh[:, k, :],
                                     AF.Identity, bias=nmr[:], scale=rstd[:])

        with ExitStack() as stg:
            wcon = stg.enter_context(tc.tile_pool(name="wcon", bufs=1))
            chk = stg.enter_context(tc.tile_pool(name="chk", bufs=4))
            chkr = stg.enter_context(tc.tile_pool(name="chkr", bufs=4))
            vtmp = stg.enter_context(tc.tile_pool(name="vtmp", bufs=3))
            cps = stg.enter_context(tc.tile_pool(name="cpsum", bufs=4, space="PSUM"))
            vps = stg.enter_context(tc.tile_pool(name="vpsum", bufs=4, space="PSUM"))

            wf_s = wcon.tile([128, 2, C], F32)
            wg_s = wcon.tile([128, 2, C], F32)
            wh_s = wcon.tile([128, 2, C], F32)
            bf_s = wcon.tile([128, 2], F32)
            bg_s = wcon.tile([128, 2], F32)
            bh_s = wcon.tile([128, C], F32)
            nc.sync.dma_start(wf_s[:], wf[:])
            nc.sync.dma_start(wg_s[:], wg[:])
            nc.sync.dma_start(wh_s[:], wh[:])
            nc.sync.dma_start(bf_s[:], bfb[:])
            nc.sync.dma_start(bg_s[:], bgb[:])
            nc.sync.dma_start(bh_s[:], bhb[:])
            wf_r = wcon.tile([128, 2, C], F32R)
            wg_r = wcon.tile([128, 2, C], F32R)
            wh_r = wcon.tile([128, 2, C], F32R)
            nc.vector.tensor_copy(wf_r[:], wf_s[:])
            nc.vector.tensor_copy(wg_r[:], wg_s[:])
            nc.vector.tensor_copy(wh_r[:], wh_s[:])

            def staged_r(dram_slice, name):
                """DMA a [128, 2, NW] chunk then round it into an f32r tile."""
                t = chk.tile([128, 2, NW], F32, tag="chk", name=name + "_f")
                nc.sync.dma_start(t[:], dram_slice)
                tr = chkr.tile([128, 2, NW], F32R, tag="chkr", name=name + "_r")
                nc.scalar.copy(tr[:], t[:])
                return tr

            wfr = wf_r[:]
            wgr = wg_r[:]
            whr = wh_r[:]

            # Fq conv (content_key shard): out[c2, n] = WfT.T @ ck + bf
            for nm in range(NM):
                tr = staged_r(ck[:, :, nm * NW:(nm + 1) * NW], f"ck{nm}")
                for c2 in range(2):
                    ps = cps.tile([128, NW], F32, tag="cps")
                    for k in range(2):
                        nc.tensor.matmul(
                            ps[:],
                            wfr[:, k, c2 * 128:(c2 + 1) * 128],
                            tr[:, k, :],
                            start=(k == 0), stop=(k == 1))
                    nc.scalar.activation(fq_s[:, c2, nm * NW:(nm + 1) * NW], ps[:],
                                         AF.Identity, bias=bf_s[:, c2:c2 + 1], scale=1.0)

            # G conv (style_key) and V conv (style), interleaved per chunk:
            # G: [c2, m] = WgT.T @ sk + bg ;  V[m, c] = st.T @ WhT + bh, V2 = V^2
            for h in range(2):
                for lm in range(NM):
                    mm = h * NM + lm
                    tg = staged_r(sk[:, h, :, lm * NW:(lm + 1) * NW], f"sk{mm}")
                    tv = staged_r(st[:, h, :, lm * NW:(lm + 1) * NW], f"st{mm}")
                    for c2 in range(2):
                        ps = cps.tile([128, NW], F32, tag="cps")
                        for k in range(2):
                            nc.tensor.matmul(
                                ps[:],
                                wgr[:, k, c2 * 128:(c2 + 1) * 128],
                                tg[:, k, :],
                                start=(k == 0), stop=(k == 1))
                        nc.scalar.activation(g_s[:, c2, mm * NW:(mm + 1) * NW], ps[:],
                                             AF.Identity, bias=bg_s[:, c2:c2 + 1], scale=1.0)
                    for sub in range(NM):
                        mt = mm * NM + sub
                        ps = vps.tile([128, C], F32, tag="vps")
                        for k in range(2):
                            nc.tensor.matmul(
                                ps[:],
                                tv[:, k, sub * 128:(sub + 1) * 128],
                                whr[:, k, :],
                                start=(k == 0), stop=(k == 1))
                        vt = vtmp.tile([128, C], F32, tag="vt", name=f"vt{mt}", bufs=2)
                        nc.vector.tensor_add(vt[:], ps[:], bh_s[:])
                        nc.vector.tensor_copy(v_s[:, mt, :], vt[:])
                        nc.vector.tensor_mul(v2_s[:, mt, :], vt[:], vt[:])

        # ---------------- flash attention inner loops ----------------
        from concourse import bass_isa

        with ExitStack() as inner:
            pt_pool = inner.enter_context(tc.tile_pool(name="pt", bufs=4))
            wrk = inner.enter_context(tc.tile_pool(name="wrk", bufs=14))
            accp = inner.enter_context(tc.tile_pool(name="accp", bufs=2))
            outp = inner.enter_context(tc.tile_pool(name="outp", bufs=4))
            lps = inner.enter_context(tc.tile_pool(name="lpsum", bufs=4, space="PSUM"))
            aps = inner.enter_context(tc.tile_pool(name="apsum", bufs=4, space="PSUM"))

            for nm in range(NM):
                nsl = slice(nm * NW, (nm + 1) * NW)
                mean_ps = [aps.tile([128, NW], F32, tag="acc", name=f"mean_ps{nm}_{i}")
                           for i in range(2)]
                e2_ps = [aps.tile([128, NW], F32, tag="acc", name=f"e2_ps{nm}_{i}")
                         for i in range(2)]
                acc = accp.tile([128, NW], F32, tag="acc", name=f"pacc{nm}")
                for mt in range(MT):
                    msl = slice(mt * 128, (mt + 1) * 128)
                    ps_l = lps.tile([128, NW], F32, tag="log")
                    for k in range(2):
                        nc.tensor.matmul(ps_l[:],
                                         g_s[:, k, msl],
                                         fq_s[:, k, nsl],
                                         start=(k == 0), stop=(k == 1))
                    pt = pt_pool.tile([128, NW], F32R, tag="pt")
                    nc.scalar.activation(pt[:], ps_l[:], AF.Exp,
                                         bias=negshift[:], scale=1.0)
                    first, last = (mt == 0), (mt == MT - 1)
                    for c2 in range(2):
                        nc.tensor.matmul(mean_ps[c2][:],
                                         v_s[:, mt, c2 * 128:(c2 + 1) * 128],
                                         pt[:], start=first, stop=last)
                    for c2 in range(2):
                        nc.tensor.matmul(e2_ps[c2][:],
                                         v2_s[:, mt, c2 * 128:(c2 + 1) * 128],
                                         pt[:], start=first, stop=last)
                    # partition-wise accumulate P for the softmax denominator
                    if first:
                        nc.vector.tensor_copy(acc[:], pt[:])
                    else:
                        nc.vector.tensor_add(acc[:], acc[:], pt[:])



                if nm == 0:
                    # Content stats slot into the nm=0 window: DMAs queue
                    # behind the style tensors; reduces fill engine slack.
                    emit_stats()

                # Eagerly drain the accumulator PSUM banks to SBUF (on ACT)
                # so the next n-macro's matmuls never wait on the reciprocal
                # chain and DVE keeps its slack for the epilogue.
                mean_sb, e2_sb = [], []
                for c2 in range(2):
                    t = wrk.tile([128, NW], F32, tag="wrk", name=f"msb{nm}_{c2}")
                    nc.scalar.copy(t[:], mean_ps[c2][:])
                    mean_sb.append(t)
                    t = wrk.tile([128, NW], F32, tag="wrk", name=f"esb{nm}_{c2}")
                    nc.scalar.copy(t[:], e2_ps[c2][:])
                    e2_sb.append(t)

                # epilogue for this n-macro: rowsum = partition-reduce(acc),
                # broadcast to all partitions by the GpSimd daisy chain.
                rsb = wrk.tile([128, NW], F32, tag="wrk", name=f"rsb{nm}")
                nc.gpsimd.partition_all_reduce(rsb[:], acc[:], channels=128,
                                               reduce_op=bass_isa.ReduceOp.add)
                recip_b = wrk.tile([128, NW], F32, tag="wrk", name=f"rcp{nm}")
                nc.vector.reciprocal(recip_b[:], rsb[:])
                meanN, e2N, var = [], [], []
                for c2 in range(2):
                    t = wrk.tile([128, NW], F32, tag="wrk", name=f"mn{nm}_{c2}")
                    nc.vector.tensor_mul(t[:], mean_sb[c2][:], recip_b[:])
                    meanN.append(t)
                    t = wrk.tile([128, NW], F32, tag="wrk", name=f"e2{nm}_{c2}")
                    nc.vector.tensor_mul(t[:], e2_sb[c2][:], recip_b[:])
                    e2N.append(t)
                for c2 in range(2):
                    sq = wrk.tile([128, NW], F32, tag="wrk", name=f"sq{nm}_{c2}")
                    nc.vector.tensor_mul(sq[:], meanN[c2][:], meanN[c2][:])
                    v = wrk.tile([128, NW], F32, tag="wrk", name=f"vr{nm}_{c2}")
                    nc.vector.tensor_sub(v[:], e2N[c2][:], sq[:])
                    nc.vector.tensor_scalar_max(v[:], v[:], 0.0)
                    var.append(v)
                stds = []
                for c2 in range(2):
                    s = wrk.tile([128, NW], F32, tag="wrk", name=f"sd{nm}_{c2}")
                    nc.scalar.sqrt(s[:], var[c2][:])
                    stds.append(s)
                for c2 in range(2):
                    ot = outp.tile([128, NW], F32, tag="out", name=f"ot{nm}_{c2}")
                    nc.vector.tensor_mul(ot[:], stds[c2][:], nrm_s[:, c2, nsl])
                    nc.vector.tensor_add(ot[:], ot[:], meanN[c2][:])
                    nc.sync.dma_start(out[:, c2, nsl], ot[:])


def kernel(content, style, content_key, style_key, Wf, bf, Wg, bg, Wh, bh):
    content = np.ascontiguousarray(np.asarray(content, dtype=np.float32))
    style = np.ascontiguousarray(np.asarray(style, dtype=np.float32))
    content_key = np.ascontiguousarray(np.asarray(content_key, dtype=np.float32))
    style_key = np.ascontiguousarray(np.asarray(style_key, dtype=np.float32))
    Wf = np.asarray(Wf, dtype=np.float32)
    Wg = np.asarray(Wg, dtype=np.float32)
    Wh = np.asarray(Wh, dtype=np.float32)
    bf = np.asarray(bf, dtype=np.float32)
    bg = np.asarray(bg, dtype=np.float32)
    bh = np.asarray(bh, dtype=np.float32)

    def wlay(W):  # [O, C] -> [128, 2, 256] with [p, k, c_out] = W[c_out, k*128+p]
        return np.ascontiguousarray(W.T.reshape(2, 128, C).transpose(1, 0, 2))

    def blay(b):  # [256] -> [128, 2]
        return np.ascontiguousarray(b.reshape(2, 128).T)

    def big_lay(x):  # [256, 4096] -> [128, 2(h), 2(k), 2048]
        return np.ascontiguousarray(
            x.reshape(2, 128, 2, NSH).transpose(1, 2, 0, 3))

    wf_l, wg_l, wh_l = wlay(Wf), wlay(Wg), wlay(Wh)
    bf_l, bg_l = blay(bf), blay(bg)
    bh_b = np.ascontiguousarray(np.broadcast_to(bh, (128, C)))

    in_maps = []
    for core in range(8):
        b, half = core // 2, core % 2
        off = half * NSH
        ctb = content[b].reshape(C, HW)
        ct_rot = np.concatenate([ctb[:, off:], ctb[:, :off]], axis=1) if off else ctb
        ck_sh = content_key[b].reshape(C, HW)[:, off:off + NSH]
        in_maps.append({
            "ck": np.ascontiguousarray(ck_sh.reshape(2, 128, NSH).transpose(1, 0, 2)),
            "ct": big_lay(ct_rot),
            "sk": big_lay(style_key[b].reshape(C, HW)),
            "st": big_lay(style[b].reshape(C, HW)),
            "wf": wf_l, "wg": wg_l, "wh": wh_l,
            "bfb": bf_l, "bgb": bg_l, "bhb": bh_b,
        })

    nc = _build_nc()
    trace = bool(os.environ.get("KERNEL_TRACE"))
    res = run_bass_kernel_spmd(nc, in_maps, core_ids=list(range(8)), trace=trace)
    _last_result.clear()
    _last_result["exec_time_ns"] = res.exec_time_ns
    _last_result["trace"] = res.instructions_and_trace

    outp = np.empty((B, C, HW), dtype=np.float32)
    for core in range(8):
        b, half = core // 2, core % 2
        o = res.results[core]["out"]          # [128, 2, NSH]
        outp[b, :, half * NSH:(half + 1) * NSH] = (
            o.transpose(1, 0, 2).reshape(C, NSH))
    return outp.reshape(B, C, 64, 64)


# revision 42
# speedup vs baseline: 1.1349x; 1.0229x over previous
"""Adaptive-style-attention (AdaAttN-like) Trainium2 kernel, 8 NeuronCores.

Math (per batch b, with N = M = 64*64 = 4096 pixels, C = Ck = 256):
  Fq = Wf @ content_key[b] + bf          # [C, N]   (q^T)
  G  = Wg @ style_key[b]   + bg          # [C, M]   (k)
  Hv = Wh @ style[b]       + bh          # [C, M];  V = Hv^T  [M, C]
  S  = softmax_m(q @ k)                  # [N, M]
  mean = S @ V ; e2 = S @ V^2            # [N, C]
  std  = sqrt(relu(e2 - mean^2))
  out  = std * mvn(content[b]) + mean    # [C, N] layout

Sharding: 8 cores = batch(4) x query-halves(2). Each core computes its
2048 query rows against the full 4096 style pixels of its batch.

Everything is computed transposed ([c, n] / [m, n] layouts) so no
on-chip transposes are needed:
  logits^T tile [m=128, n=512] = G_chunk.T @ Fq_chunk   (K = c)
  P^T = exp(logits^T - SHIFT)  (global shift; logits ~ N(0, 256); the
        actual global max logit is ~97, exp(97-48) fits fp32 easily)
  mean^T [c, n] += (V[m, c])-as-lhsT @ P^T  (K = m), PSUM-accumulated
  rowsum via elementwise P accumulation on DVE + one GpSimd
        partition_all_reduce per n-macro (frees the TensorengIne), the
        all-reduce also broadcasts, so 1/rowsum needs no extra matmul.
  out = std * normc + mean.

All matmuls run in float32r: on TRN2 silicon f32r streams the moving
operand at 2 cycles/row (~400 ns per 128x128x512 matmul) with the fused
4-byte weight load fully hidden, and carries ~14-bit mantissa accuracy
(probe: rms 2.3e-3 on K=256 N(0,16^2) logits, 5x better than tf32
emulation).  bf16 matmuls measure the SAME ~380-400 ns here (the per-
matmul LDWEIGHTS cannot hide under a 216 ns stream and FWL is disabled
in this toolchain), so bf16 gives no speed advantage and costs softmax
accuracy - f32r everywhere is optimal.  The BIR verifier requires f32r
matmul operands to be produced rounded, hence compute-engine rounding
copies on every DMA-staged conv input.
"""
import os
import numpy as np

import concourse.bass as bass
import concourse.mybir as mybir
import concourse.tile as tile
from concourse import bacc
from concourse.bass_utils import run_bass_kernel_spmd

B, C, HW = 4, 256, 64 * 64          # N = M = HW
NSH = HW // 2                        # queries per core = 2048
SHIFT = 48.0
EPS = 1e-5
F32 = mybir.dt.float32
F32R = mybir.dt.float32r
BF16 = mybir.dt.bfloat16
AF = mybir.ActivationFunctionType

_last_result = {}


def _build_nc() -> bass.Bass:
    nc = bacc.Bacc("TRN2", target_bir_lowering=False)
    ck = nc.dram_tensor("ck", [128, 2, NSH], F32, kind="ExternalInput")        # content_key shard [p, kchunk, n]
    ct = nc.dram_tensor("ct", [128, 2, 2, NSH], F32, kind="ExternalInput")     # content (rotated) [p, half, kchunk, n]
    sk = nc.dram_tensor("sk", [128, 2, 2, NSH], F32, kind="ExternalInput")     # style_key [p, half, kchunk, m]
    st = nc.dram_tensor("st", [128, 2, 2, NSH], F32, kind="ExternalInput")     # style     [p, half, kchunk, m]
    wf = nc.dram_tensor("wf", [128, 2, C], F32, kind="ExternalInput")          # Wf^T [p(ch), chunk, c_out]
    wg = nc.dram_tensor("wg", [128, 2, C], F32, kind="ExternalInput")
    wh = nc.dram_tensor("wh", [128, 2, C], F32, kind="ExternalInput")
    bfb = nc.dram_tensor("bfb", [128, 2], F32, kind="ExternalInput")           # bf [p, c-chunk]
    bgb = nc.dram_tensor("bgb", [128, 2], F32, kind="ExternalInput")
    bhb = nc.dram_tensor("bhb", [128, C], F32, kind="ExternalInput")           # bh broadcast over partitions
    out = nc.dram_tensor("out", [128, 2, NSH], F32, kind="ExternalOutput")     # [p, c-chunk, n]

    with tile.TileContext(nc) as tc:
        _emit(nc, tc, ck, ct, sk, st, wf, wg, wh, bfb, bgb, bhb, out)
    nc.compile()
    return nc


def _emit(nc, tc, ck, ct, sk, st, wf, wg, wh, bfb, bgb, bhb, out):
    from contextlib import ExitStack

    NM = 4          # n macro tiles of 512 within the 2048-query shard
    MT = 32         # m tiles of 128 within 4096 style pixels
    NW = 512

    with ExitStack() as persist:
        consts = persist.enter_context(tc.tile_pool(name="consts", bufs=1))
        fq_p = persist.enter_context(tc.tile_pool(name="fq", bufs=1))
        g_p = persist.enter_context(tc.tile_pool(name="g", bufs=1))
        v_p = persist.enter_context(tc.tile_pool(name="v", bufs=1))
        v2_p = persist.enter_context(tc.tile_pool(name="v2", bufs=1))
        nrm_p = persist.enter_context(tc.tile_pool(name="nrm", bufs=1))

        negshift = consts.tile([128, 1], F32)
        nc.vector.memset(negshift[:], -SHIFT)

        fq_s = fq_p.tile([128, 2, NSH], F32R)     # [p, c-chunk, n]
        g_s = g_p.tile([128, 2, HW], F32R)        # [p, c-chunk, m]
        v_s = v_p.tile([128, MT, C], F32R)        # [p(m), m-tile, c]
        v2_s = v2_p.tile([128, MT, C], F32R)
        nrm_s = nrm_p.tile([128, 2, NSH], F32)    # normalized content [p, c-chunk, n]

        # ---------------- staging + convs (stats deferred) ----------------
        # Conv inputs stream through small chunk pools: DMA (f32) -> ACT
        # rounding copy (-> f32r) -> matmuls.  G and V convs interleave so
        # both style tensors stream concurrently.  Content stats are emitted
        # later (inside the first attention n-macro) so their DMA + reduce
        # work does not compete with kernel startup.
        ct_pool = persist.enter_context(tc.tile_pool(name="ctsh", bufs=1))
        schk = persist.enter_context(tc.tile_pool(name="schk", bufs=2))
        sml = persist.enter_context(tc.tile_pool(name="sml", bufs=24))
        prt = persist.enter_context(tc.tile_pool(name="prt", bufs=1))
        sqd = persist.enter_context(tc.tile_pool(name="sqd", bufs=2))

        def emit_stats():
            ct_sh = ct_pool.tile([128, 2, NSH], F32)
            nc.sync.dma_start(ct_sh[:], ct[:, 0, :, :])
            parts_s = prt.tile([128, 2, 8], F32)    # Σx partials   [p, k, col]
            parts_q = prt.tile([128, 2, 8], F32)    # Σx² partials
            col = 0
            for c in range(NM):
                for k in range(2):
                    sl = ct_sh[:, k, c * NW:(c + 1) * NW]
                    nc.vector.reduce_sum(parts_s[:, k, col:col + 1], sl,
                                         axis=mybir.AxisListType.X)
                    dump = sqd.tile([128, NW], F32, tag="sqd", name=f"sqa{c}_{k}")
                    nc.scalar.activation(dump[:], sl, AF.Square,
                                         accum_out=parts_q[:, k, col:col + 1])
                col += 1
            for c in range(NM):
                t = schk.tile([128, 2, NW], F32, tag="schk", name=f"cth{c}")
                nc.sync.dma_start(t[:], ct[:, 1, :, c * NW:(c + 1) * NW])
                for k in range(2):
                    nc.vector.reduce_sum(parts_s[:, k, col:col + 1], t[:, k, :],
                                         axis=mybir.AxisListType.X)
                    dump = sqd.tile([128, NW], F32, tag="sqd", name=f"sqb{c}_{k}")
                    nc.scalar.activation(dump[:], t[:, k, :], AF.Square,
                                         accum_out=parts_q[:, k, col:col + 1])
                col += 1
            for k in range(2):
                s_all = sml.tile([128, 1], F32, tag="sml", name=f"sa{k}")
                ss_all = sml.tile([128, 1], F32, tag="sml", name=f"ssa{k}")
                nc.vector.reduce_sum(s_all[:], parts_s[:, k, :], axis=mybir.AxisListType.X)
                nc.vector.reduce_sum(ss_all[:], parts_q[:, k, :], axis=mybir.AxisListType.X)
                m_t = sml.tile([128, 1], F32, tag="sml", name=f"m{k}")
                nc.vector.tensor_scalar_mul(m_t[:], s_all[:], 1.0 / HW)
                msq = sml.tile([128, 1], F32, tag="sml", name=f"msq{k}")
                nc.vector.tensor_mul(msq[:], s_all[:], m_t[:])       # (Σx)²/N
                var = sml.tile([128, 1], F32, tag="sml", name=f"va{k}")
                nc.vector.tensor_sub(var[:], ss_all[:], msq[:])
                nc.vector.tensor_scalar_mul(var[:], var[:], 1.0 / (HW - 1))
                nc.vector.tensor_scalar_add(var[:], var[:], EPS)
                sd = sml.tile([128, 1], F32, tag="sml", name=f"sd{k}")
                nc.scalar.sqrt(sd[:], var[:])
                rstd = sml.tile([128, 1], F32, tag="sml", name=f"rs{k}")
                nc.vector.reciprocal(rstd[:], sd[:])
                nmr = sml.tile([128, 1], F32, tag="sml", name=f"nm{k}")
                nc.vector.tensor_mul(nmr[:], m_t[:], rstd[:])
                nc.vector.tensor_scalar_mul(nmr[:], nmr[:], -1.0)
                nc.scalar.activation(nrm_s[:, k, :], ct_sh[:, k, :],
                                     AF.Identity, bias=nmr[:], scale=rstd[:])

        with ExitStack() as stg:
            wcon = stg.enter_context(tc.tile_pool(name="wcon", bufs=1))
            chk = stg.enter_context(tc.tile_pool(name="chk", bufs=4))
            chkr = stg.enter_context(tc.tile_pool(name="chkr", bufs=4))
            vtmp = stg.enter_context(tc.tile_pool(name="vtmp", bufs=3))
            cps = stg.enter_context(tc.tile_pool(name="cpsum", bufs=4, space="PSUM"))
            vps = stg.enter_context(tc.tile_pool(name="vpsum", bufs=4, space="PSUM"))

            wf_s = wcon.tile([128, 2, C], F32)
            wg_s = wcon.tile([128, 2, C], F32)
            wh_s = wcon.tile([128, 2, C], F32)
            bf_s = wcon.tile([128, 2], F32)
            bg_s = wcon.tile([128, 2], F32)
            bh_s = wcon.tile([128, C], F32)
            nc.sync.dma_start(wf_s[:], wf[:])
            nc.sync.dma_start(wg_s[:], wg[:])
            nc.sync.dma_start(wh_s[:], wh[:])
            nc.sync.dma_start(bf_s[:], bfb[:])
            nc.sync.dma_start(bg_s[:], bgb[:])
            nc.sync.dma_start(bh_s[:], bhb[:])
            wf_r = wcon.tile([128, 2, C], F32R)
            wg_r = wcon.tile([128, 2, C], F32R)
            wh_r = wcon.tile([128, 2, C], F32R)
            nc.vector.tensor_copy(wf_r[:], wf_s[:])
            nc.vector.tensor_copy(wg_r[:], wg_s[:])
            nc.vector.tensor_copy(wh_r[:], wh_s[:])

            def staged_r(dram_slice, name):
                """DMA a [128, 2, NW] chunk then round it into an f32r tile."""
                t = chk.tile([128, 2, NW], F32, tag="chk", name=name + "_f")
                nc.sync.dma_start(t[:], dram_slice)
                tr = chkr.tile([128, 2, NW], F32R, tag="chkr", name=name + "_r")
                nc.scalar.copy(tr[:], t[:])
                return tr

            wfr = wf_r[:]
            wgr = wg_r[:]
            whr = wh_r[:]

            # Fq conv (content_key shard): out[c2, n] = WfT.T @ ck + bf
            for nm in range(NM):
                tr = staged_r(ck[:, :, nm * NW:(nm + 1) * NW], f"ck{nm}")
                for c2 in range(2):
                    ps = cps.tile([128, NW], F32, tag="cps")
                    for k in range(2):
                        nc.tensor.matmul(
                            ps[:],
                            wfr[:, k, c2 * 128:(c2 + 1) * 128],
                            tr[:, k, :],
                            start=(k == 0), stop=(k == 1))
                    nc.scalar.activation(fq_s[:, c2, nm * NW:(nm + 1) * NW], ps[:],
                                         AF.Identity, bias=bf_s[:, c2:c2 + 1], scale=1.0)

            # G conv (style_key) and V conv (style), interleaved per chunk:
            # G: [c2, m] = WgT.T @ sk + bg ;  V[m, c] = st.T @ WhT + bh, V2 = V^2
            for h in range(2):
                for lm in range(NM):
                    mm = h * NM + lm
                    tg = staged_r(sk[:, h, :, lm * NW:(lm + 1) * NW], f"sk{mm}")
                    tv = staged_r(st[:, h, :, lm * NW:(lm + 1) * NW], f"st{mm}")
                    for c2 in range(2):
                        ps = cps.tile([128, NW], F32, tag="cps")
                        for k in range(2):
                            nc.tensor.matmul(
                                ps[:],
                                wgr[:, k, c2 * 128:(c2 + 1) * 128],
                                tg[:, k, :],
                                start=(k == 0), stop=(k == 1))
                        nc.scalar.activation(g_s[:, c2, mm * NW:(mm + 1) * NW], ps[:],
                                             AF.Identity, bias=bg_s[:, c2:c2 + 1], scale=1.0)
                    for sub in range(NM):
                        mt = mm * NM + sub
                        ps = vps.tile([128, C], F32, tag="vps")
                        for k in range(2):
                            nc.tensor.matmul(
                                ps[:],
                                tv[:, k, sub * 128:(sub + 1) * 128],
                                whr[:, k, :],
                                start=(k == 0), stop=(k == 1))
                        vt = vtmp.tile([128, C], F32, tag="vt", name=f"vt{mt}", bufs=2)
                        nc.vector.tensor_add(vt[:], ps[:], bh_s[:])
                        nc.vector.tensor_copy(v_s[:, mt, :], vt[:])
                        nc.vector.tensor_mul(v2_s[:, mt, :], vt[:], vt[:])

        # ---------------- flash attention inner loops ----------------
        from concourse import bass_isa

        with ExitStack() as inner:
            pt_pool = inner.enter_context(tc.tile_pool(name="pt", bufs=4))
            wrk = inner.enter_context(tc.tile_pool(name="wrk", bufs=14))
            accp = inner.enter_context(tc.tile_pool(name="accp", bufs=2))
            outp = inner.enter_context(tc.tile_pool(name="outp", bufs=4))
            lps = inner.enter_context(tc.tile_pool(name="lpsum", bufs=4, space="PSUM"))
            aps = inner.enter_context(tc.tile_pool(name="apsum", bufs=4, space="PSUM"))

            for nm in range(NM):
                nsl = slice(nm * NW, (nm + 1) * NW)
                mean_ps = [aps.tile([128, NW], F32, tag="acc", name=f"mean_ps{nm}_{i}")
                           for i in range(2)]
                e2_ps = [aps.tile([128, NW], F32, tag="acc", name=f"e2_ps{nm}_{i}")
                         for i in range(2)]
                acc = accp.tile([128, NW], F32, tag="acc", name=f"pacc{nm}")
                for mt in range(MT):
                    msl = slice(mt * 128, (mt + 1) * 128)
                    ps_l = lps.tile([128, NW], F32, tag="log")
                    for k in range(2):
                        nc.tensor.matmul(ps_l[:],
                                         g_s[:, k, msl],
                                         fq_s[:, k, nsl],
                                         start=(k == 0), stop=(k == 1))
                    pt = pt_pool.tile([128, NW], F32R, tag="pt")
                    nc.scalar.activation(pt[:], ps_l[:], AF.Exp,
                                         bias=negshift[:], scale=1.0)
                    first, last = (mt == 0), (mt == MT - 1)
                    for c2 in range(2):
                        nc.tensor.matmul(mean_ps[c2][:],
                                         v_s[:, mt, c2 * 128:(c2 + 1) * 128],
                                         pt[:], start=first, stop=last)
                    for c2 in range(2):
                        nc.tensor.matmul(e2_ps[c2][:],
                                         v2_s[:, mt, c2 * 128:(c2 + 1) * 128],
                                         pt[:], start=first, stop=last)
                    # partition-wise accumulate P for the softmax denominator
                    if first:
                        nc.vector.tensor_copy(acc[:], pt[:])
                    else:
                        nc.vector.tensor_add(acc[:], acc[:], pt[:])



                if nm == 0:
                    # Content stats slot into the nm=0 window: DMAs queue
                    # behind the style tensors; reduces fill engine slack.
                    emit_stats()

                # Eagerly drain the accumulator PSUM banks to SBUF (on ACT)
                # so the next n-macro's matmuls never wait on the reciprocal
                # chain and DVE keeps its slack for the epilogue.
                mean_sb, e2_sb = [], []
                for c2 in range(2):
                    t = wrk.tile([128, NW], F32, tag="wrk", name=f"msb{nm}_{c2}")
                    nc.scalar.copy(t[:], mean_ps[c2][:])
                    mean_sb.append(t)
                    t = wrk.tile([128, NW], F32, tag="wrk", name=f"esb{nm}_{c2}")
                    nc.vector.tensor_copy(t[:], e2_ps[c2][:])
                    e2_sb.append(t)

                # epilogue for this n-macro: rowsum = partition-reduce(acc),
                # broadcast to all partitions by the GpSimd daisy chain.
                rsb = wrk.tile([128, NW], F32, tag="wrk", name=f"rsb{nm}")
                nc.gpsimd.partition_all_reduce(rsb[:], acc[:], channels=128,
                                               reduce_op=bass_isa.ReduceOp.add)
                recip_b = wrk.tile([128, NW], F32, tag="wrk", name=f"rcp{nm}")
                nc.vector.reciprocal(recip_b[:], rsb[:])
                meanN, e2N, var = [], [], []
                for c2 in range(2):
                    t = wrk.tile([128, NW], F32, tag="wrk", name=f"mn{nm}_{c2}")
                    nc.vector.tensor_mul(t[:], mean_sb[c2][:], recip_b[:])
                    meanN.append(t)
                    t = wrk.tile([128, NW], F32, tag="wrk", name=f"e2{nm}_{c2}")
                    nc.vector.tensor_mul(t[:], e2_sb[c2][:], recip_b[:])
                    e2N.append(t)
                for c2 in range(2):
                    sq = wrk.tile([128, NW], F32, tag="wrk", name=f"sq{nm}_{c2}")
                    nc.vector.tensor_mul(sq[:], meanN[c2][:], meanN[c2][:])
                    v = wrk.tile([128, NW], F32, tag="wrk", name=f"vr{nm}_{c2}")
                    nc.vector.tensor_sub(v[:], e2N[c2][:], sq[:])
                    nc.vector.tensor_scalar_max(v[:], v[:], 0.0)
                    var.append(v)
                stds = []
                for c2 in range(2):
                    s = wrk.tile([128, NW], F32, tag="wrk", name=f"sd{nm}_{c2}")
                    nc.scalar.sqrt(s[:], var[c2][:])
                    stds.append(s)
                for c2 in range(2):
                    ot = outp.tile([128, NW], F32, tag="out", name=f"ot{nm}_{c2}")
                    nc.vector.tensor_mul(ot[:], stds[c2][:], nrm_s[:, c2, nsl])
                    nc.vector.tensor_add(ot[:], ot[:], meanN[c2][:])
                    nc.sync.dma_start(out[:, c2, nsl], ot[:])


def kernel(content, style, content_key, style_key, Wf, bf, Wg, bg, Wh, bh):
    content = np.ascontiguousarray(np.asarray(content, dtype=np.float32))
    style = np.ascontiguousarray(np.asarray(style, dtype=np.float32))
    content_key = np.ascontiguousarray(np.asarray(content_key, dtype=np.float32))
    style_key = np.ascontiguousarray(np.asarray(style_key, dtype=np.float32))
    Wf = np.asarray(Wf, dtype=np.float32)
    Wg = np.asarray(Wg, dtype=np.float32)
    Wh = np.asarray(Wh, dtype=np.float32)
    bf = np.asarray(bf, dtype=np.float32)
    bg = np.asarray(bg, dtype=np.float32)
    bh = np.asarray(bh, dtype=np.float32)

    def wlay(W):  # [O, C] -> [128, 2, 256] with [p, k, c_out] = W[c_out, k*128+p]
        return np.ascontiguousarray(W.T.reshape(2, 128, C).transpose(1, 0, 2))

    def blay(b):  # [256] -> [128, 2]
        return np.ascontiguousarray(b.reshape(2, 128).T)

    def big_lay(x):  # [256, 4096] -> [128, 2(h), 2(k), 2048]
        return np.ascontiguousarray(
            x.reshape(2, 128, 2, NSH).transpose(1, 2, 0, 3))

    wf_l, wg_l, wh_l = wlay(Wf), wlay(Wg), wlay(Wh)
    bf_l, bg_l = blay(bf), blay(bg)
    bh_b = np.ascontiguousarray(np.broadcast_to(bh, (128, C)))

    in_maps = []
    for core in range(8):
        b, half = core // 2, core % 2
        off = half * NSH
        ctb = content[b].reshape(C, HW)
        ct_rot = np.concatenate([ctb[:, off:], ctb[:, :off]], axis=1) if off else ctb
        ck_sh = content_key[b].reshape(C, HW)[:, off:off + NSH]
        in_maps.append({
            "ck": np.ascontiguousarray(ck_sh.reshape(2, 128, NSH).transpose(1, 0, 2)),
            "ct": big_lay(ct_rot),
            "sk": big_lay(style_key[b].reshape(C, HW)),
            "st": big_lay(style[b].reshape(C, HW)),
            "wf": wf_l, "wg": wg_l, "wh": wh_l,
            "bfb": bf_l, "bgb": bg_l, "bhb": bh_b,
        })

    nc = _build_nc()
    trace = bool(os.environ.get("KERNEL_TRACE"))
    res = run_bass_kernel_spmd(nc, in_maps, core_ids=list(range(8)), trace=trace)
    _last_result.clear()
    _last_result["exec_time_ns"] = res.exec_time_ns
    _last_result["trace"] = res.instructions_and_trace

    outp = np.empty((B, C, HW), dtype=np.float32)
    for core in range(8):
        b, half = core // 2, core % 2
        o = res.results[core]["out"]          # [128, 2, NSH]
        outp[b, :, half * NSH:(half + 1) * NSH] = (
            o.transpose(1, 0, 2).reshape(C, NSH))
    return outp.reshape(B, C, 64, 64)
